# revision 19
# baseline (speedup 1.0000x reference)
"""Trainium2 Bass kernel for AttnDecoderBlock (window attention + MLP + bilinear upsample).

v2: data-parallel B=128 -> 8 cores x 16 samples.  Feature-major on-chip layout
[C_partition, token_free].  LN affine folded into GEMM weights on host.
Linear GEMMs (qkv, v, proj, fc1, fc2) run in fp8e4m3 with DoubleRow perf mode
(two K-tiles per matmul pass); attention S/O and the final out GEMM stay bf16.
The trunk (X, t1) is bf16.  LN stats accumulate across samples into one PSUM
tile via per-sample selector columns; LN row math is batched (all 16 samples
for LN1, per-group bounce into a batched tile for LN2).  Structure:
load+LN1-stats phase -> attention pass (8 groups of 2) -> LN2 row math ->
MLP+output pass.  Bilinear upsample runs on scalar(scales, relu folded in) +
vector(adds) + gpsimd(edges).
"""

import numpy as np
import ml_dtypes
from contextlib import ExitStack

from concourse import bacc, mybir
import concourse.bass as bass
import concourse.tile as tile

dt = mybir.dt
BF = dt.bfloat16
F32 = dt.float32
F8 = dt.float8e4
AF = mybir.ActivationFunctionType
OP = mybir.AluOpType
DR = mybir.MatmulPerfMode.DoubleRow

B, C, NH, WS, H, W = 128, 384, 6, 20, 15, 20
HD = C // NH            # 64
L = H * W               # 300 real tokens
N = WS * WS             # 400 padded tokens
OUT_DIM, OUT_H, OUT_W = 192, 30, 40
N_CORES = 8
S = B // N_CORES        # 16 samples per core
G = 2                   # samples per group
GT = G * L              # 600
TP = 304                # padded per-sample token stride (DoubleRow step%16==0)
GT8 = G * TP            # 608
bf16 = ml_dtypes.bfloat16
f8e4 = ml_dtypes.float8_e4m3fn


def _rel_pos_index(ws):
    coords = np.stack(np.meshgrid(np.arange(ws), np.arange(ws), indexing='ij')).reshape(2, -1)
    rel = (coords[:, :, None] - coords[:, None, :]).transpose(1, 2, 0)
    rel[:, :, 0] += ws - 1
    rel[:, :, 1] += ws - 1
    rel[:, :, 0] *= 2 * ws - 1
    return rel.sum(-1)


def build_consts(ln1_s, ln1_b, qkv_w, qkv_b, bias_table, proj_w, proj_b,
                 ln2_s, ln2_b, fc1_w, fc1_b, fc2_w, fc2_b, out_w, out_b):
    f32 = np.float32
    qkv_w = np.asarray(qkv_w, f32); qkv_b = np.asarray(qkv_b, f32)
    ln1_s = np.asarray(ln1_s, f32); ln1_b = np.asarray(ln1_b, f32)
    Wq = ln1_s[:, None] * qkv_w[:, 0:C] * (HD ** -0.5)
    Wk = ln1_s[:, None] * qkv_w[:, C:2 * C]
    Wv = ln1_s[:, None] * qkv_w[:, 2 * C:3 * C]
    bq = (ln1_b @ qkv_w[:, 0:C] + qkv_b[0:C]) * (HD ** -0.5)
    bk = ln1_b @ qkv_w[:, C:2 * C] + qkv_b[C:2 * C]
    bv = ln1_b @ qkv_w[:, 2 * C:] + qkv_b[2 * C:]
    assert not np.any(bq) and not np.any(bk) and not np.any(bv), \
        "nonzero qkv/ln1 bias path not implemented"
    Wqk = np.concatenate([Wq, Wk], axis=1)            # [C, 768]
    W1 = np.asarray(ln2_s, f32)[:, None] * np.asarray(fc1_w, f32)
    b1 = np.asarray(ln2_b, f32) @ np.asarray(fc1_w, f32) + np.asarray(fc1_b, f32)
    assert not np.any(b1) and not np.any(proj_b) and not np.any(fc2_b) and not np.any(out_b), \
        "nonzero bias path not implemented"

    REL = _rel_pos_index(WS)
    bias = np.asarray(bias_table, f32)[REL].transpose(2, 0, 1)   # [NH, 400, 400]
    EB_T = np.exp(bias[:, :L, :L].transpose(0, 2, 1))            # [NH, j, i]
    EBp = np.zeros((NH, 384, L), f32)
    EBp[:, :L] = EB_T
    PB = np.exp(bias[:, :L, L:]).sum(-1)                         # [NH, 300]

    def pad4(w, F):  # [C,F] -> [4,128,F] fp8 with zero 4th K-chunk
        z = np.zeros((4, 128, F), f32)
        z[0:3] = w.reshape(3, 128, F)
        return z.astype(f8e4)

    esel2 = np.zeros((NH, 3 * 128), f32)
    for f in range(3):
        esel2[2 * f, f * 128:f * 128 + 64] = 1.0
        esel2[2 * f + 1, f * 128 + 64:(f + 1) * 128] = 1.0
    e16 = np.zeros((S, S * 128), f32)
    for s in range(S):
        e16[s, s * 128:(s + 1) * 128] = 1.0
    ecol = np.zeros((128, S, S), f32)
    for s in range(S):
        ecol[:, s, s] = 1.0

    c = {}
    c['wqk'] = pad4(Wqk, 2 * C)
    c['wv'] = pad4(Wv, C)
    c['wp'] = pad4(np.asarray(proj_w, f32), C)
    c['w1'] = pad4(W1, 4 * C)
    c['w2'] = np.ascontiguousarray(np.asarray(fc2_w, f32).reshape(12, 128, C)).astype(f8e4)
    c['wo'] = np.ascontiguousarray(np.asarray(out_w, f32).reshape(3, 128, OUT_DIM)).astype(bf16)
    c['eb'] = np.ascontiguousarray(EBp.reshape(NH, 3, 128, L)).astype(bf16)
    c['pb'] = PB.astype(f32)
    c['esel2'] = esel2.astype(bf16)
    c['e16'] = e16.astype(bf16)
    c['ecolb'] = ecol.astype(bf16)
    return c


CONST_SPECS = [
    ('wqk', (4, 128, 2 * C), F8), ('wv', (4, 128, C), F8), ('wp', (4, 128, C), F8),
    ('w1', (4, 128, 4 * C), F8), ('w2', (12, 128, C), F8), ('wo', (3, 128, OUT_DIM), BF),
    ('eb', (NH, 3, 128, L), BF), ('pb', (NH, L), F32),
    ('esel2', (NH, 3 * 128), BF), ('e16', (S, S * 128), BF),
    ('ecolb', (128, S, S), BF),
]


def build_program(n_samples, debug=False):
    assert n_samples == S
    nc = bacc.Bacc(None, target_bir_lowering=False, debug=debug)
    xin = nc.dram_tensor("xin", [n_samples, 3, 128, L], F32, kind="ExternalInput")
    outd = nc.dram_tensor("out", [n_samples, OUT_DIM, OUT_H, OUT_W], F32,
                          kind="ExternalOutput")
    cdram = {name: nc.dram_tensor(name, list(shape), d, kind="ExternalInput")
             for name, shape, d in CONST_SPECS}

    n_groups = n_samples // G

    with tile.TileContext(nc) as tc, ExitStack() as ctx:
        cpool = ctx.enter_context(tc.tile_pool(name="consts", bufs=1))
        pers = ctx.enter_context(tc.tile_pool(name="pers", bufs=1))
        pool = ctx.enter_context(tc.tile_pool(name="main", bufs=1))
        ps = ctx.enter_context(tc.tile_pool(name="psum", bufs=1, space="PSUM"))

        cs = {}
        for name, shape, d in CONST_SPECS:
            if len(shape) == 2:
                t = cpool.tile([shape[0], shape[1]], d, tag=name, name=name)
                nc.sync.dma_start(t[:], cdram[name][:])
            elif name == 'eb':
                t = cpool.tile([128, NH * 3 * L], d, tag=name, name=name)
                nc.sync.dma_start(t.rearrange("p (h j i) -> p h j i", h=NH, j=3),
                                  cdram[name].rearrange("h j p i -> p h j i"))
            elif name == 'ecolb':
                t = cpool.tile([128, S * S], d, tag=name, name=name)
                nc.sync.dma_start(t.rearrange("p (a b) -> p a b", a=S), cdram[name][:])
            else:  # [k, 128, F] weight stacks
                k, p, f = shape
                t = cpool.tile([128, k * f], d, tag=name, name=name)
                nc.sync.dma_start(t.rearrange("p (k f) -> p k f", k=k),
                                  cdram[name].rearrange("k p f -> p k f"))
            cs[name] = t

        def w4(name):
            return cs[name].rearrange("p (k f) -> p k f", k=4)

        vecolb = cs['ecolb'].rearrange("p (a b) -> p a b", a=S)

        # persistent activations (bf16 trunk)
        Xb = pers.tile([128, 3 * S * L], BF, tag="Xb", name="Xb")
        vXb = Xb.rearrange("p (c s t) -> p c s t", c=3, s=S)
        t1 = pers.tile([128, 3 * S * L], BF, tag="t1", name="t1")
        vt1a = t1.rearrange("p (c s t) -> p c s t", c=3, s=S)
        st1 = pers.tile([S, 2 * L], F32, tag="st1", name="st1")
        st2 = pers.tile([S, 2 * L], F32, tag="st2", name="st2")

        def ln_rows(st, tag):
            """batched mean/rstd over [S, L]; returns bf16 [S, L] mean, rstd."""
            m = pool.tile([S, L], F32, tag="lr_m", name=f"{tag}_m", bufs=2)
            q = pool.tile([S, L], F32, tag="lr_q", name=f"{tag}_q", bufs=2)
            y = pool.tile([S, L], F32, tag="lr_y", name=f"{tag}_y", bufs=2)
            t0 = pool.tile([S, L], F32, tag="lr_t0", name=f"{tag}_t0", bufs=2)
            nc.vector.tensor_scalar(m[:], st[:, 0:L], 1.0 / C, None, OP.mult)
            nc.vector.tensor_scalar(q[:], st[:, L:2 * L], 1.0 / C, 1e-5, OP.mult, OP.add)
            nc.vector.tensor_tensor(t0[:], m[:], m[:], OP.mult)
            nc.vector.tensor_tensor(q[:], q[:], t0[:], OP.subtract)   # var+eps
            nc.vector.tensor_scalar(y[:], q[:], -0.5, 1.5, OP.mult, OP.add)
            for _ in range(2):  # newton: y = y*(1.5 - 0.5*v*y*y)
                nc.vector.tensor_tensor(t0[:], y[:], y[:], OP.mult)
                nc.vector.tensor_tensor(t0[:], t0[:], q[:], OP.mult)
                nc.vector.tensor_scalar(t0[:], t0[:], -0.5, 1.5, OP.mult, OP.add)
                nc.vector.tensor_tensor(y[:], y[:], t0[:], OP.mult)
            mb = pool.tile([S, L], BF, tag="lr_mb", name=f"{tag}_mb", bufs=2)
            rb = pool.tile([S, L], BF, tag="lr_rb", name=f"{tag}_rb", bufs=2)
            nc.vector.tensor_copy(mb[:], m[:])
            nc.vector.tensor_copy(rb[:], y[:])
            return mb, rb

        # ================= phase 1: load X + LN1 stats (all samples) ==========
        ps_sum = ps.tile([128, 512], F32, tag="psB", name="ps_sum", bufs=4)
        ps_sq = ps.tile([128, 512], F32, tag="psB", name="ps_sq", bufs=4)
        for s in range(n_samples):
            XF = pool.tile([128, 3 * L], F32, tag="XFl", name="XFl", bufs=3)
            vXF = XF.rearrange("p (c t) -> p c t", c=3)
            for c0 in range(3):
                nc.sync.dma_start(vXF[:, c0, :], xin[s, c0])
            nc.scalar.activation(vXb[:, :, s, :], vXF[:, :, :], AF.Copy)
            sq = pool.tile([128, 3 * L], BF, tag="sq1", name="sq1", bufs=3)
            vsq = sq.rearrange("p (c t) -> p c t", c=3)
            nc.vector.tensor_tensor(vsq[:, :, :], vXb[:, :, s, :], vXb[:, :, s, :], OP.mult)
            for c0 in range(3):
                nc.tensor.matmul(ps_sum[0:S, 0:L], vecolb[:, s, :], vXb[:, c0, s, :],
                                 start=(s == 0 and c0 == 0),
                                 stop=(s == n_samples - 1 and c0 == 2),
                                 skip_group_check=True)
                nc.tensor.matmul(ps_sq[0:S, 0:L], vecolb[:, s, :], vsq[:, c0, :],
                                 start=(s == 0 and c0 == 0),
                                 stop=(s == n_samples - 1 and c0 == 2),
                                 skip_group_check=True)
        nc.vector.tensor_copy(st1[:, 0:L], ps_sum[0:S, 0:L])
        nc.vector.tensor_copy(st1[:, L:2 * L], ps_sq[0:S, 0:L])
        mb1, rb1 = ln_rows(st1, "ln1")

        # ================= pass A: attention per group =======================
        for g in range(n_groups):
            xh = pool.tile([128, 4 * GT8], F8, tag="xh", name="xh", bufs=2)
            vxh = xh.rearrange("p (k s t) -> p k s t", k=4, s=G)
            for sl in range(G):
                s = g * G + sl
                psm = ps.tile([128, 512], F32, tag="psB", name="psm", bufs=4)
                psr = ps.tile([128, 512], F32, tag="psB", name="psr", bufs=4)
                nc.tensor.matmul(psm[:, 0:L], cs['e16'][:, s * 128:(s + 1) * 128], mb1[:, :])
                nc.tensor.matmul(psr[:, 0:L], cs['e16'][:, s * 128:(s + 1) * 128], rb1[:, :])
                tmp = pool.tile([128, 3 * L], BF, tag="lntmp", name="lntmp", bufs=2)
                vtmp = tmp.rearrange("p (c t) -> p c t", c=3)
                for c0 in range(3):
                    nc.vector.tensor_tensor(vtmp[:, c0, :], vXb[:, c0, s, :],
                                            psm[:, 0:L], OP.subtract)
                    nc.vector.tensor_tensor(vxh[:, c0, sl, 0:L], vtmp[:, c0, :],
                                            psr[:, 0:L], OP.mult)

            # qk GEMM fp8 DoubleRow -> qk bf16 (feature-major)
            qk = [pool.tile([128, GT], BF, tag=f"qk{f}", name=f"qk{f}", bufs=2)
                  for f in range(6)]
            for f in range(6):
                psg = ps.tile([128, 1024], F32, tag="psA", name="psA", bufs=2)
                for sl in range(G):
                    nc.tensor.matmul(psg[:, sl * 512:sl * 512 + L],
                                     w4('wqk')[:, 0:2, f * 128:(f + 1) * 128],
                                     vxh[:, 0:2, sl, 0:L],
                                     start=True, stop=False, perf_mode=DR)
                    nc.tensor.matmul(psg[:, sl * 512:sl * 512 + L],
                                     w4('wqk')[:, 2, f * 128:(f + 1) * 128],
                                     vxh[:, 2, sl, 0:L],
                                     start=False, stop=True)
                vps = psg.rearrange("p (s c) -> p s c", s=2)[:, :, 0:L]
                if f >= 3:
                    nc.scalar.activation(qk[f].rearrange("p (s c) -> p s c", s=G), vps,
                                         AF.Copy)
                else:
                    nc.vector.tensor_copy(qk[f].rearrange("p (s c) -> p s c", s=G), vps)

            # v GEMM fp8 DR (token-major out) -> vv bf16 [100, G,3,NH,65]
            vT = pool.tile([128, G * 3 * (NH * 65)], BF, tag="vT", name="vT", bufs=1)
            vv = vT.rearrange("p (s t h c) -> p s t h c", s=G, t=3, h=NH)
            for sl in range(G):
                for tcn, (ta, tw) in enumerate([(0, 128), (128, 128), (256, 44)]):
                    psv = ps.tile([128, 512], F32, tag="psB", name="psv", bufs=4)
                    nc.tensor.matmul(psv[0:tw, 0:C],
                                     vxh[:, 0:2, sl, ta:ta + tw],
                                     w4('wv')[:, 0:2, :],
                                     start=True, stop=False, perf_mode=DR)
                    nc.tensor.matmul(psv[0:tw, 0:C],
                                     vxh[:, 2, sl, ta:ta + tw],
                                     w4('wv')[:, 2, :],
                                     start=False, stop=True)
                    pv = psv[:, 0:C].rearrange("p (h c) -> p h c", h=NH)[0:tw, :, 0:64]
                    nc.scalar.activation(vv[0:tw, sl, tcn, :, 0:64], pv, AF.Copy)
                    nc.gpsimd.memset(vv[0:tw, sl, tcn, :, 64:65], 1.0)

            # attention
            O = pool.tile([128, 4 * GT8], F8, tag="O", name="O", bufs=2)
            vO = O.rearrange("p (k s t) -> p k s t", k=4, s=G)
            for sl in range(G):
                dband = pool.tile([128, NH * L], BF, tag="dband", name="dband", bufs=1)
                o_un = pool.tile([128, 3 * L], BF, tag="o_un", name="o_un", bufs=3)
                for h in range(NH):
                    fq, pq = h // 2, (h % 2) * 64
                    psS = ps.tile([128, 1024], F32, tag="psA", name="psA", bufs=2)
                    psS3 = ps.tile([128, 512], F32, tag="psB", name="psS3", bufs=4)
                    for jc, (ta, tw) in enumerate([(0, 128), (128, 128), (256, 44)]):
                        dst = psS[0:tw, jc * 512:jc * 512 + L] if jc < 2 else \
                            psS3[0:tw, 0:L]
                        nc.tensor.matmul(
                            dst,
                            qk[3 + fq][pq:pq + 64, sl * L + ta: sl * L + ta + tw],
                            qk[fq][pq:pq + 64, sl * L:(sl + 1) * L])
                    PT2 = pool.tile([128, 3 * L], BF, tag="PT2", name="PT2", bufs=3)
                    vPT2 = PT2.rearrange("p (j c) -> p j c", j=3)
                    nc.scalar.activation(vPT2[:, 0:2, :],
                                         psS.rearrange("p (j c) -> p j c", j=2)[:, :, 0:L],
                                         AF.Exp)
                    nc.scalar.activation(vPT2[0:44, 2, :], psS3[0:44, 0:L], AF.Exp)
                    nc.vector.tensor_tensor(
                        PT2[:, 0:2 * L], PT2[:, 0:2 * L],
                        cs['eb'][:, (h * 3) * L:(h * 3 + 2) * L], OP.mult)
                    nc.gpsimd.tensor_tensor(
                        PT2[0:44, 2 * L:3 * L], PT2[0:44, 2 * L:3 * L],
                        cs['eb'][0:44, (h * 3 + 2) * L:(h * 3 + 3) * L], OP.mult)
                    psO = ps.tile([128, 512], F32, tag="psB", name="psO", bufs=4)
                    for jc, (ta, tw) in enumerate([(0, 128), (128, 128), (256, 44)]):
                        nc.tensor.matmul(psO[0:65, 0:L],
                                         vv[0:tw, sl, jc, h, :],
                                         PT2[0:tw, jc * L:(jc + 1) * L],
                                         start=(jc == 0), stop=(jc == 2))
                    if pq == 0:
                        nc.scalar.activation(o_un[0:64, fq * L:(fq + 1) * L],
                                             psO[0:64, 0:L], AF.Copy)
                    else:
                        scr = pool.tile([64, L], BF, tag="oscr", name="oscr", bufs=3)
                        nc.scalar.activation(scr[:, :], psO[0:64, 0:L], AF.Copy)
                        nc.sync.dma_start(o_un[64:128, fq * L:(fq + 1) * L], scr[:, :])
                    if pq == 0:
                        nc.vector.tensor_copy(dband[64:65, h * L:(h + 1) * L],
                                              psO[64:65, 0:L])
                    else:
                        nc.scalar.activation(dband[64:65, h * L:(h + 1) * L],
                                             psO[64:65, 0:L], AF.Copy)
                rinv_raw = pool.tile([NH, L], BF, tag="rinv_raw", name="rinv_raw", bufs=1)
                nc.sync.dma_start(rinv_raw.rearrange("h (o i) -> h o i", o=1),
                                  dband[64:65].rearrange("p (h i) -> p h i", h=NH))
                rinv_g = pool.tile([NH, L], F32, tag="rinv_g", name="rinv_g", bufs=1)
                nc.vector.tensor_tensor(rinv_g[:], rinv_raw[:], cs['pb'][:, :], OP.add)
                rinv_f = pool.tile([NH, L], F32, tag="rinv_f", name="rinv_f", bufs=1)
                nc.vector.reciprocal_approx_fast(rinv_f[:], rinv_g[:])
                rinv_b = pool.tile([NH, L], BF, tag="rinv_b", name="rinv_b", bufs=1)
                nc.vector.tensor_copy(rinv_b[:], rinv_f[:])
                for f in range(3):
                    psR = ps.tile([128, 512], F32, tag="psB", name="psR", bufs=4)
                    nc.tensor.matmul(psR[:, 0:L], cs['esel2'][:, f * 128:(f + 1) * 128],
                                     rinv_b[:, :])
                    nc.vector.tensor_tensor(vO[:, f, sl, 0:L], o_un[:, f * L:(f + 1) * L],
                                            psR[:, 0:L], OP.mult)

            # proj fp8 DR + shortcut -> t1 bf16; LN2 stats
            for f in range(3):
                psg = ps.tile([128, 1024], F32, tag="psA", name="psA", bufs=2)
                for sl in range(G):
                    nc.tensor.matmul(psg[:, sl * 512:sl * 512 + L],
                                     w4('wp')[:, 0:2, f * 128:(f + 1) * 128],
                                     vO[:, 0:2, sl, 0:L],
                                     start=True, stop=False, perf_mode=DR)
                    nc.tensor.matmul(psg[:, sl * 512:sl * 512 + L],
                                     w4('wp')[:, 2, f * 128:(f + 1) * 128],
                                     vO[:, 2, sl, 0:L],
                                     start=False, stop=True)
                vps = psg.rearrange("p (s c) -> p s c", s=2)[:, :, 0:L]
                nc.vector.tensor_tensor(vt1a[:, f, g * G:(g + 1) * G, :], vps,
                                        vXb[:, f, g * G:(g + 1) * G, :], OP.add)
            sq2 = pool.tile([128, 3 * GT], BF, tag="sq2", name="sq2", bufs=1)
            vsq2 = sq2.rearrange("p (c s t) -> p c s t", c=3, s=G)
            nc.vector.tensor_tensor(vsq2[:, :, :, :], vt1a[:, :, g * G:(g + 1) * G, :],
                                    vt1a[:, :, g * G:(g + 1) * G, :], OP.mult)
            ps_s2 = ps.tile([128, 512], F32, tag="psB", name="ps_s2", bufs=4)
            ps_q2 = ps.tile([128, 512], F32, tag="psB", name="ps_q2", bufs=4)
            for sl in range(G):
                s = g * G + sl
                for c0 in range(3):
                    nc.tensor.matmul(ps_s2[0:G, 0:L],
                                     vecolb[:, s, g * G:(g + 1) * G],
                                     vt1a[:, c0, s, :],
                                     start=(sl == 0 and c0 == 0), stop=(sl == 1 and c0 == 2),
                                     skip_group_check=True)
                    nc.tensor.matmul(ps_q2[0:G, 0:L],
                                     vecolb[:, s, g * G:(g + 1) * G],
                                     vsq2[:, c0, sl, :],
                                     start=(sl == 0 and c0 == 0), stop=(sl == 1 and c0 == 2),
                                     skip_group_check=True)
            stb = pool.tile([G, 2 * L], F32, tag="stb", name="stb", bufs=2)
            nc.vector.tensor_copy(stb[:, 0:L], ps_s2[0:G, 0:L])
            nc.vector.tensor_copy(stb[:, L:2 * L], ps_q2[0:G, 0:L])
            nc.sync.dma_start(st2[g * G:(g + 1) * G, :], stb[:, :])

        mb2, rb2 = ln_rows(st2, "ln2")

        # ================= pass B: MLP + out + upsample ======================
        for g in range(n_groups):
            xh2 = pool.tile([128, 4 * GT8], F8, tag="xh2", name="xh2", bufs=2)
            vxh2 = xh2.rearrange("p (k s t) -> p k s t", k=4, s=G)
            for sl in range(G):
                s = g * G + sl
                psm = ps.tile([128, 512], F32, tag="psB", name="psm", bufs=4)
                psr = ps.tile([128, 512], F32, tag="psB", name="psr", bufs=4)
                nc.tensor.matmul(psm[:, 0:L], cs['e16'][:, s * 128:(s + 1) * 128], mb2[:, :])
                nc.tensor.matmul(psr[:, 0:L], cs['e16'][:, s * 128:(s + 1) * 128], rb2[:, :])
                tmp = pool.tile([128, 3 * L], BF, tag="lntmp", name="lntmp", bufs=2)
                vtmp = tmp.rearrange("p (c t) -> p c t", c=3)
                for c0 in range(3):
                    nc.vector.tensor_tensor(vtmp[:, c0, :], vt1a[:, c0, s, :],
                                            psm[:, 0:L], OP.subtract)
                    nc.vector.tensor_tensor(vxh2[:, c0, sl, 0:L], vtmp[:, c0, :],
                                            psr[:, 0:L], OP.mult)

            # fc1 fp8 DR -> gelu -> fc1h fp8
            fc1h = pool.tile([128, 12 * GT8], F8, tag="fc1h", name="fc1h", bufs=1)
            vfc1 = fc1h.rearrange("p (k s t) -> p k s t", k=12, s=G)
            for f in range(12):
                psg = ps.tile([128, 1024], F32, tag="psA", name="psA", bufs=2)
                for sl in range(G):
                    nc.tensor.matmul(psg[:, sl * 512:sl * 512 + L],
                                     w4('w1')[:, 0:2, f * 128:(f + 1) * 128],
                                     vxh2[:, 0:2, sl, 0:L],
                                     start=True, stop=False, perf_mode=DR)
                    nc.tensor.matmul(psg[:, sl * 512:sl * 512 + L],
                                     w4('w1')[:, 2, f * 128:(f + 1) * 128],
                                     vxh2[:, 2, sl, 0:L],
                                     start=False, stop=True)
                vps = psg.rearrange("p (s c) -> p s c", s=2)[:, :, 0:L]
                nc.scalar.activation(vfc1[:, f, :, 0:L], vps, AF.Gelu)

            # fc2 fp8 DR (K=1536, 6 pairs) + residual -> t2 bf16
            vw2 = cs['w2'].rearrange("p (k f) -> p k f", k=12)
            t2 = [pool.tile([128, GT], BF, tag=f"t2_{f}", name=f"t2_{f}", bufs=2)
                  for f in range(3)]
            for f in range(3):
                psg = ps.tile([128, 1024], F32, tag="psA", name="psA", bufs=2)
                for sl in range(G):
                    for p2 in range(6):
                        nc.tensor.matmul(psg[:, sl * 512:sl * 512 + L],
                                         vw2[:, 2 * p2:2 * p2 + 2, f * 128:(f + 1) * 128],
                                         vfc1[:, 2 * p2:2 * p2 + 2, sl, 0:L],
                                         start=(p2 == 0), stop=(p2 == 5), perf_mode=DR)
                vps = psg.rearrange("p (s c) -> p s c", s=2)[:, :, 0:L]
                nc.vector.tensor_tensor(t2[f].rearrange("p (s c) -> p s c", s=G),
                                        vps, vt1a[:, f, g * G:(g + 1) * G, :], OP.add)

            # out GEMM bf16; relu folded into upsample scales
            vwo = cs['wo'].rearrange("p (k f) -> p k f", k=3)
            for f in range(2):
                fw = 128 if f == 0 else 64
                psg = ps.tile([128, 1024], F32, tag="psA", name="psA", bufs=2)
                for sl in range(G):
                    for k in range(3):
                        nc.tensor.matmul(psg[0:fw, sl * 512:sl * 512 + L],
                                         vwo[:, k, f * 128:f * 128 + fw],
                                         t2[k][:, sl * L:(sl + 1) * L],
                                         start=(k == 0), stop=(k == 2))
                PC = fw
                vps = psg.rearrange("p (s c) -> p s c", s=2)[0:PC, :, 0:L]
                p25 = pool.tile([128, GT], BF, tag="p25", name="p25", bufs=1)
                p75 = pool.tile([128, GT], BF, tag="p75", name="p75", bufs=1)
                v25 = p25.rearrange("p (s y x) -> p s y x", s=G, y=15)
                v75 = p75.rearrange("p (s y x) -> p s y x", s=G, y=15)
                nc.scalar.activation(p25.rearrange("p (s c) -> p s c", s=2)[0:PC],
                                     vps, AF.Relu, 0.0, 0.25)
                nc.scalar.activation(p75.rearrange("p (s c) -> p s c", s=2)[0:PC],
                                     vps, AF.Relu, 0.0, 0.75)
                XI = pool.tile([128, 2 * GT], BF, tag="XI", name="XI", bufs=1)
                vXI = XI.rearrange("p (s y x t) -> p s y x t", s=G, y=15, x=20)
                nc.gpsimd.tensor_tensor(vXI[0:PC, :, :, 1:20, 0:1], v25[0:PC, :, :, 0:19],
                                        v75[0:PC, :, :, 1:20], OP.add)
                nc.gpsimd.tensor_tensor(vXI[0:PC, :, :, 0:1, 0:1], v25[0:PC, :, :, 0:1],
                                        v75[0:PC, :, :, 0:1], OP.add)
                nc.gpsimd.tensor_tensor(vXI[0:PC, :, :, 0:19, 1:2], v75[0:PC, :, :, 0:19],
                                        v25[0:PC, :, :, 1:20], OP.add)
                nc.gpsimd.tensor_tensor(vXI[0:PC, :, :, 19:20, 1:2], v75[0:PC, :, :, 19:20],
                                        v25[0:PC, :, :, 19:20], OP.add)
                q25 = pool.tile([128, 2 * GT], BF, tag="q25", name="q25", bufs=1)
                q75 = pool.tile([128, 2 * GT], BF, tag="q75", name="q75", bufs=1)
                nc.vector.tensor_scalar(q25[0:PC, :], XI[0:PC, :], 0.25, None, OP.mult)
                nc.vector.tensor_scalar(q75[0:PC, :], XI[0:PC, :], 0.75, None, OP.mult)
                EY = pool.tile([128, 2 * GT], F32, tag="EY", name="EY", bufs=1)
                OY = pool.tile([128, 2 * GT], F32, tag="OY", name="OY", bufs=1)
                vEY = EY.rearrange("p (s y x) -> p s y x", s=G, y=15)
                vOY = OY.rearrange("p (s y x) -> p s y x", s=G, y=15)
                vq25 = q25.rearrange("p (s y x) -> p s y x", s=G, y=15)
                vq75 = q75.rearrange("p (s y x) -> p s y x", s=G, y=15)
                vXI2 = XI.rearrange("p (s y x) -> p s y x", s=G, y=15)
                nc.vector.tensor_tensor(vEY[0:PC, :, 1:15, :], vq25[0:PC, :, 0:14, :],
                                        vq75[0:PC, :, 1:15, :], OP.add)
                nc.vector.tensor_copy(vEY[0:PC, :, 0:1, :], vXI2[0:PC, :, 0:1, :])
                nc.vector.tensor_tensor(vOY[0:PC, :, 0:14, :], vq75[0:PC, :, 0:14, :],
                                        vq25[0:PC, :, 1:15, :], OP.add)
                nc.vector.tensor_copy(vOY[0:PC, :, 14:15, :], vXI2[0:PC, :, 14:15, :])
                for sl in range(G):
                    ov = outd[g * G + sl, f * 128:f * 128 + PC].rearrange(
                        "c (y t) x -> c y (t x)", t=2)
                    nc.sync.dma_start(ov[:, :, 0:40], vEY[0:PC, sl, :, :])
                    nc.sync.dma_start(ov[:, :, 40:80], vOY[0:PC, sl, :, :])

    nc.compile()
    return nc


_PROG_CACHE = {}


def kernel(x, ln1_s, ln1_b, qkv_w, qkv_b, bias_table, proj_w, proj_b,
           ln2_s, ln2_b, fc1_w, fc1_b, fc2_w, fc2_b, out_w, out_b):
    from concourse.bass_utils import run_bass_kernel_spmd
    x = np.asarray(x, np.float32)
    consts = build_consts(ln1_s, ln1_b, qkv_w, qkv_b, bias_table, proj_w, proj_b,
                          ln2_s, ln2_b, fc1_w, fc1_b, fc2_w, fc2_b, out_w, out_b)
    if S not in _PROG_CACHE:
        _PROG_CACHE[S] = build_program(S)
    nc = _PROG_CACHE[S]
    xs = x.reshape(B, 3, 128, H * W)
    in_maps = []
    for cid in range(N_CORES):
        m = {'xin': np.ascontiguousarray(xs[cid * S:(cid + 1) * S])}
        m.update(consts)
        in_maps.append(m)
    res = run_bass_kernel_spmd(nc, in_maps, core_ids=list(range(N_CORES)))
    out = np.concatenate([r['out'] for r in res.results], axis=0)
    return out.astype(np.float32)
